# revision 24
# baseline (speedup 1.0000x reference)
"""Vocab-parallel fused log_softmax(x @ W^T) for one TRN2 chip (8 NeuronCores).

Strategy (tensor-parallel over vocab, per the sharding hint):
  - W^T is sharded over vocab across 8 cores (6284 cols each, 50257 padded
    to 50272; the 15 pad columns live on core 7 and contribute exp(0)=1,
    removed via the Ln bias).
  - fp8 e4m3 matmuls in MatmulPerfMode.DoubleRow: each instruction consumes
    2 k-tiles of 128 (lhsT [128,2,128], rhs [128,2,nw]), measured ~0.52
    ns/output-col on HW (~2x the fp32r rate). W is pre-scaled by 32 on the
    host so its sigma~0.7 sits in e4m3's normal range; the 1/32 rescale is
    folded into the Exp activation scale and the final fused
    (raw*1/32 - logZ) tensor_scalar. Measured absmax/scale ~ 1.38e-2
    (gate 2e-2); error is host-side quantization, fully deterministic.
  - W shard resident in SBUF as fp8 (12.9 MB, DMA'd once, k-pair
    interleaved host layout: w8[p, kp, n, i]); x packed per 256-token
    chunk (0.5 MB fp8, x8[ci*128+p, kk, t]).
  - 16 chunks x 2 m-tiles x 13 n-tiles; each PSUM tile accumulates 8
    DoubleRow matmuls; n-tiles sweep in groups of 4 so 4 banks drain
    (DVE cast to bf16 logits in SBUF) while 4 accumulate. One Exp per
    m-tile over the staged bf16 logits gives the per-token sum via
    accum_out. Per-chunk AllReduce (HBM bounce) of [128, 2] sums.
  - The logZ -> subtract -> DMA-out chain is deferred TWO chunks: one
    core starts ~90us late (axon launch stagger), and anything reading
    an AllReduce result would otherwise head-of-line-block the FIFO
    engine queues and stall the PE at every chunk boundary.
  - Output written bf16 (values |x| <= ~17, abs err <= 0.03), host upcasts.

Measured: ~0.99 ms NEFF span on core 0 (MM stream 833 us busy / 96%
occupancy; tail is the straggler core's lag surfacing at the last
AllReduce). Baseline fp32r kernel: 2.29 ms.
"""

import numpy as np
import ml_dtypes

import concourse.bacc as bacc
import concourse.mybir as mybir
from concourse import tile
from concourse.bass_utils import run_bass_kernel_spmd

F32 = mybir.dt.float32
BF16 = mybir.dt.bfloat16
F8 = mybir.dt.float8e4
AF = mybir.ActivationFunctionType
ALU = mybir.AluOpType
NP_F8 = ml_dtypes.float8_e4m3fn

VOCAB = 50257
D = 2048
TOKENS = 4096
N_CORES = 8
V_SHARD = 6284                      # padded vocab columns per core
PAD = N_CORES * V_SHARD - VOCAB     # 15 zero columns, all on core 7
N_SIZES = [256, 396] + [512] * 11   # n-tile split, small first
assert sum(N_SIZES) == V_SHARD
CHUNK = 256                         # tokens per pipeline chunk
MT = CHUNK // 128                   # m-tiles per chunk
KT = D // 128                       # 16 k-tiles of 128
KP = KT // 2                        # 8 DoubleRow steps
N_CHUNKS = TOKENS // CHUNK
WSCALE = 32.0                       # host premultiplier on W before e4m3 cast


def build_nc(n_cores=N_CORES):
    nt = len(N_SIZES)
    inv = 1.0 / WSCALE

    nc = bacc.Bacc("TRN2", target_bir_lowering=False, debug=False,
                   num_devices=n_cores)
    x8 = nc.dram_tensor("x8", [N_CHUNKS * 128, KT, CHUNK], F8,
                        kind="ExternalInput").ap()
    w8 = nc.dram_tensor("w8", [128, KP, V_SHARD, 2], F8,
                        kind="ExternalInput").ap()
    out = nc.dram_tensor("out", [TOKENS, V_SHARD], BF16,
                         kind="ExternalOutput").ap()

    # n-tile groups of <=4: one group's 4 PSUM banks accumulate while the
    # previous group's 4 drain; within a group the stationary x tile is
    # identical across the ni sweep (walrus can skip redundant LDWEIGHTS)
    groups = []
    g = []
    for ni in range(nt):
        g.append(ni)
        if len(g) == 4:
            groups.append(g)
            g = []
    if g:
        groups.append(g)
    n_offsets = np.concatenate([[0], np.cumsum(N_SIZES)]).tolist()

    with tile.TileContext(nc) as tc:
        with tc.tile_pool(name="wp", bufs=1) as wp, \
             tc.tile_pool(name="xp", bufs=2) as xp, \
             tc.tile_pool(name="lp", bufs=1) as lp, \
             tc.tile_pool(name="sp", bufs=2) as sp, \
             tc.tile_pool(name="dp", bufs=2) as dpool, \
             tc.tile_pool(name="ps", bufs=8, space="PSUM") as ps, \
             tc.tile_pool(name="dram", bufs=4, space="DRAM") as dram:
            padbias = sp.tile([128, 1], F32, tag="padbias", bufs=1)
            nc.vector.memset(padbias[:], -float(PAD))

            # warmup AllReduce: align the 8 cores before the pipeline
            warm_in = dram.tile([128, 1], F32, tag="warm_in", bufs=1,
                                name="warm_in")
            warm_out = dram.tile([128, 1], F32, tag="warm_out", bufs=1,
                                 addr_space="Shared", name="warm_out")
            nc.gpsimd.dma_start(warm_in[:], padbias[:])
            nc.gpsimd.collective_compute(
                "AllReduce", ALU.add,
                replica_groups=[list(range(n_cores))],
                ins=[warm_in.opt()], outs=[warm_out.opt()])

            # chunk 0's x first so the PE isn't stuck behind the full W DMA
            xa0 = xp.tile([128, KT, CHUNK], F8, tag="xa", name="xa_0")
            nc.sync.dma_start(xa0[:], x8[0:128, :, :])

            # resident fp8 W shard, one DMA per n-tile
            wts = []
            for ni, nw in enumerate(N_SIZES):
                wt = wp.tile([128, KP, nw, 2], F8, tag=f"w{ni}",
                             name=f"w_{ni}")
                nc.sync.dma_start(wt[:], w8[:, :, n_offsets[ni]:
                                            n_offsets[ni] + nw, :])
                wts.append(wt)

            def finish_out(ci, lgs, gs):
                """Deferred two chunks: logZ = ln(sum - PAD), subtract in
                place, DMA out."""
                logz = sp.tile([128, MT], F32, tag="logz", bufs=3,
                               name=f"logz_{ci}")
                nc.scalar.activation(logz[:], gs[:], AF.Ln, bias=padbias[:])
                for mi, lg in enumerate(lgs):
                    nc.vector.tensor_scalar(
                        lg[:], lg[:], inv, logz[:, mi:mi + 1],
                        ALU.mult, ALU.subtract)
                    nc.sync.dma_start(
                        out[ci * CHUNK + mi * 128:
                            ci * CHUNK + (mi + 1) * 128, :],
                        lg[:])

            pending = []
            for ci in range(N_CHUNKS):
                if ci == 0:
                    xa = xa0
                else:
                    xa = xp.tile([128, KT, CHUNK], F8, tag="xa",
                                 name=f"xa_{ci}")
                    nc.sync.dma_start(xa[:], x8[ci * 128:(ci + 1) * 128, :, :])

                lgs = []
                ssum = sp.tile([128, MT], F32, tag="ssum", bufs=3,
                               name=f"ssum_{ci}")
                for mi in range(MT):
                    lg = lp.tile([128, V_SHARD], BF16, tag=f"lg{mi}", bufs=3,
                                 name=f"lg_{ci}_{mi}")
                    lgs.append(lg)
                    for gi, grp in enumerate(groups):
                        pts = [ps.tile([128, N_SIZES[ni]], F32, tag="ps",
                                       name=f"ps_{ci}_{mi}_{ni}")
                               for ni in grp]
                        for kp in range(KP):
                            for pt, ni in zip(pts, grp):
                                nc.tensor.matmul(
                                    pt[:],
                                    xa[:, 2 * kp:2 * kp + 2,
                                       mi * 128:(mi + 1) * 128],
                                    wts[ni][:, kp, 0:N_SIZES[ni], 0:2]
                                    .rearrange("p n two -> p two n"),
                                    start=(kp == 0), stop=(kp == KP - 1),
                                    perf_mode=mybir.MatmulPerfMode.DoubleRow)
                        for pt, ni in zip(pts, grp):
                            nc.vector.tensor_copy(
                                lg[:, n_offsets[ni]:n_offsets[ni + 1]], pt[:])

                    # one big exp over the staged (32x) logits; accum gives
                    # this m-tile's per-token sum-exp directly
                    dump = dpool.tile([128, V_SHARD], F8, tag="dump",
                                      bufs=1, name=f"dump_{ci}_{mi}")
                    nc.scalar.activation(
                        dump[:], lg[:], AF.Exp, scale=inv,
                        accum_out=ssum[:, mi:mi + 1])

                # AllReduce the chunk's per-token sums across the 8 cores
                ar_in = dram.tile([128, MT], F32, tag="ar_in",
                                  name=f"ar_in_{ci}")
                ar_out = dram.tile([128, MT], F32, tag="ar_out",
                                   addr_space="Shared", name=f"ar_out_{ci}")
                nc.gpsimd.dma_start(ar_in[:], ssum[:])
                nc.gpsimd.collective_compute(
                    "AllReduce", ALU.add,
                    replica_groups=[list(range(n_cores))],
                    ins=[ar_in.opt()], outs=[ar_out.opt()])
                gs = sp.tile([128, MT], F32, tag="gs", bufs=3, name=f"gs_{ci}")
                nc.gpsimd.dma_start(gs[:], ar_out[:])

                pending.append((ci, lgs, gs))
                # two-chunk deferral: the AR result lags ~1.5 chunks (one
                # core starts late), so anything reading it must trail far
                # enough to never head-of-line-block a FIFO queue
                if len(pending) > 2:
                    finish_out(*pending.pop(0))
            while pending:
                finish_out(*pending.pop(0))

    nc.compile()
    return nc


def _shard_inputs(x, w):
    """x: [T, D] f32, w: [V, D] f32 -> per-core in_maps (host prep)."""
    # x8[ci*128+p, kk, t] = x[ci*CHUNK + t, kk*128 + p], cast e4m3
    xq = x.astype(NP_F8)
    x8 = np.ascontiguousarray(
        xq.reshape(N_CHUNKS, CHUNK, KT, 128).transpose(0, 3, 2, 1)
    ).reshape(N_CHUNKS * 128, KT, CHUNK)
    in_maps = []
    for c in range(N_CORES):
        v0 = c * V_SHARD
        real = min(V_SHARD, VOCAB - v0)
        wsh = np.zeros((V_SHARD, D), dtype=np.float32)
        wsh[:real] = w[v0:v0 + real] * WSCALE
        q = wsh.astype(NP_F8)  # [VS, D]
        # w8[p, kp, n, i] = 32*W[v0+n, (2kp+i)*128+p]
        w8 = np.ascontiguousarray(q.T.reshape(KP, 2, 128, V_SHARD)
                                  .transpose(2, 0, 3, 1))
        in_maps.append({"x8": x8, "w8": w8})
    return in_maps


def _gather_output(results):
    full = np.empty((TOKENS, VOCAB), dtype=np.float32)
    for c in range(N_CORES):
        lo = c * V_SHARD
        hi = min(lo + V_SHARD, VOCAB)
        full[:, lo:hi] = results[c]["out"][:, :hi - lo].astype(np.float32)
    return full


_NC_CACHE = {}


def _get_nc():
    if "nc" not in _NC_CACHE:
        _NC_CACHE["nc"] = build_nc()
    return _NC_CACHE["nc"]


def kernel(input, target, proj_weight):
    x = np.asarray(input, dtype=np.float32)
    w = np.asarray(proj_weight, dtype=np.float32)
    nc = _get_nc()
    in_maps = _shard_inputs(x, w)
    res = run_bass_kernel_spmd(nc, in_maps, core_ids=list(range(N_CORES)))
    return _gather_output(res.results)
